# revision 7
# baseline (speedup 1.0000x reference)
"""Bass/Trainium2 kernel for nn_LogitsProcessorWithPacked.

Computes out[t, :] = weight_stacked[indices[t]] @ hidden_states[t]
 (T=64 tokens, H=2048 hidden, V=32000 vocab, D=4 stacked deltas, fp32).

Strategy (per sharding hint): shard weight_stacked along the vocab dim
across the 8 cores (column-parallel LM head, 4000 vocab rows per core),
replicate hidden_states/indices, gather partial logits along vocab on the
host.

Host-side prep (cheap, O(bytes) layout work only — all FLOPs run on device):
  * indices -> per-delta masks; build masked-transposed hidden HmT
    [D*H, T] and pack it into the SBUF partition layout [128, 64*64].
  * per-core weight slice [D, 4000, H] -> transposed chunk-major layout
    [64, 128, 4000] (chunk c = (d, h-block), partition p = h within block)
    so each chunk DMA is fully contiguous 16KB-per-partition lines.

Device kernel (per core): stream the 131MB of W^T through SBUF with
double-buffered 4MB DMAs; for each chunk c the PE accumulates
  acc_j[t, v'] += HmT_chunk_c.T @ WT_chunk_c[:, j-block]
into 8 PSUM-bank accumulators (one per 500-wide vocab block), fp32 PSUM.
This is memory(HBM)-bound: ~131MB / ~3.5e11 B/s ~ 380us per core.
"""

import numpy as np
from concurrent.futures import ThreadPoolExecutor

from concourse import bacc, mybir, tile
from concourse import bass_utils

# Problem constants (hardcoded per contract)
T = 64          # tokens
H = 2048        # hidden
V = 32000       # vocab
D = 4           # stacked deltas
NCORES = 8
VC = V // NCORES            # 4000 vocab rows per core
NCHUNK = D * H // 128       # 64 chunks of 128 contraction rows
VBLK = 500                  # vocab block per PSUM bank (500*4B = 2000B <= 2KB bank)
NJ = VC // VBLK             # 8 vocab blocks
NJ2 = NJ // 2               # psum accumulators (2 vocab blocks share one, via
                            # PE column-tiling: col groups 0-63 / 64-127)

# chunks per DMA / weight buffering, per dtype size (8MB transfers, double
# buffered: 128KB/partition of SBUF for weights; PE drains a buffer ~4x
# faster than DMA fills one, so depth 2 keeps the DMA engines saturated)
_DMA_PLAN = {4: (4, 2), 2: (8, 2)}  # dtype bytes -> (CPD, WBUFS)

# Numeric mode: "f32" exact (PE 4 cyc/row), "f32r" full-rate fp32 (HW reduced
# precision), "bf16x3"/"f16x3" hi/lo-split (3 products, ~1e-5 rel err,
# fp32-rate memory), "bf16"/"f16" single-pass (half memory traffic).
MODE = "f32r"

_cache = {}


def _mm_dtype(mode):
    return {
        "f32": mybir.dt.float32,
        "f32r": mybir.dt.float32r,
        "bf16": mybir.dt.bfloat16,
        "bf16x3": mybir.dt.bfloat16,
        "f16": mybir.dt.float16,
        "f16x3": mybir.dt.float16,
    }[mode]


def _nsplit(mode):
    return 2 if mode in ("bf16x3", "f16x3") else 1


def _build(mode):
    """Build + compile the per-core Bass module (SPMD: same NEFF, 8 cores)."""
    dt = _mm_dtype(mode)
    f32 = mybir.dt.float32
    nsplit = _nsplit(mode)  # hi(/lo) weight streams

    nc = bacc.Bacc("TRN2", target_bir_lowering=False, debug=False,
                   num_devices=NCORES)

    # hmt packs nsplit copies (hi, lo) of the masked-transposed hidden
    hmt_d = nc.dram_tensor("hmt", [128, nsplit, NCHUNK * T], dt,
                           kind="ExternalInput")
    wt_d = nc.dram_tensor("wt", [nsplit, NCHUNK, 128, VC], dt,
                          kind="ExternalInput")
    out_d = nc.dram_tensor("out", [T, VC], f32, kind="ExternalOutput")

    CPD, WBUFS = _DMA_PLAN[4 if dt in (f32, mybir.dt.float32r) else 2]

    with tile.TileContext(nc) as tc:
        with (
            tc.tile_pool(name="const", bufs=1) as const_pool,
            tc.tile_pool(name="wpool", bufs=WBUFS) as wpool,
            tc.tile_pool(name="accp", bufs=1, space="PSUM") as accp,
            tc.tile_pool(name="opool", bufs=1) as opool,
        ):
            hmt_sb = const_pool.tile([128, nsplit, NCHUNK * T], dt, name="hmt_sb")
            nc.sync.dma_start(hmt_sb[:], hmt_d[:])

            # 8 PSUM-bank accumulators, one per 500-wide vocab block.
            # (PE column-tiling two blocks into one [128, VBLK] bank was tried
            # and is rejected by this toolchain: walrus asserts
            # s3d3_mm_valid_dst_partition for matmul dst base_partition=64.)
            accs = [
                accp.tile([T, VBLK], f32, tag=f"acc{j}", name=f"acc{j}")
                for j in range(NJ)
            ]
            out_sb = opool.tile([T, VC], f32, name="out_sb")

            n_mm = NCHUNK * nsplit  # accumulation group length per acc
            for s in range(nsplit):
                for cc in range(NCHUNK // CPD):
                    wt_t = wpool.tile([128, CPD, VC], dt, tag="wt", name="wt_t")
                    nc.sync.dma_start(
                        wt_t[:],
                        wt_d[s, cc * CPD:(cc + 1) * CPD].rearrange("k p v -> p k v"),
                    )
                    for k in range(CPD):
                        c = cc * CPD + k
                        mi = s * NCHUNK + c
                        for j in range(NJ):
                            rhs = wt_t[:, k, j * VBLK:(j + 1) * VBLK]
                            if nsplit == 2 and s == 0:
                                # products 1+2: (hmt_hi + hmt_lo) x wt_hi
                                for part in range(2):
                                    nc.tensor.matmul(
                                        accs[j][:],
                                        lhsT=hmt_sb[:, part, c * T:(c + 1) * T],
                                        rhs=rhs,
                                        start=(c == 0 and part == 0),
                                        stop=False,
                                    )
                            else:
                                # f32/f32r/bf16/f16: one product per chunk.
                                # x3 modes s==1: product 3: hmt_hi x wt_lo
                                nc.tensor.matmul(
                                    accs[j][:],
                                    lhsT=hmt_sb[:, 0, c * T:(c + 1) * T],
                                    rhs=rhs,
                                    start=(mi == 0),
                                    stop=(mi == n_mm - 1),
                                )
            for j in range(NJ):
                nc.vector.tensor_copy(out_sb[:, j * VBLK:(j + 1) * VBLK], accs[j][:])
            nc.sync.dma_start(out_d[:], out_sb[:])

    nc.compile()
    return nc


def _np_dtype(mode):
    if mode in ("bf16", "bf16x3"):
        import ml_dtypes
        return ml_dtypes.bfloat16
    if mode in ("f16", "f16x3"):
        return np.float16
    return np.float32


def _prep_hmt(hidden_states, indices, mode):
    """[128, nsplit, NCHUNK*T]: masked transposed hidden in partition layout."""
    masks = (indices[None, :] == np.arange(D, dtype=np.int32)[:, None])  # [D, T]
    # HmT[d*H + h, t] = H[t, h] * mask[d, t]
    hmt = (hidden_states.T[None, :, :] * masks[:, None, :]).reshape(D * H, T)
    # chunk-major partition packing: [NCHUNK, 128, T] -> [128, NCHUNK*T]
    packed32 = np.ascontiguousarray(
        hmt.reshape(NCHUNK, 128, T).transpose(1, 0, 2)
    ).reshape(128, NCHUNK * T)
    nsplit = _nsplit(mode)
    ndt = _np_dtype(mode)
    out = np.zeros((128, nsplit, NCHUNK * T), dtype=ndt)
    hi = packed32.astype(ndt)
    out[:, 0] = hi
    if nsplit == 2:
        out[:, 1] = (packed32 - hi.astype(np.float32)).astype(ndt)
    return out


def _prep_wt(weight_stacked, mode):
    """[NCORES][nsplit, NCHUNK, 128, VC] transposed chunk-major weight shards."""
    nsplit = _nsplit(mode)
    ndt = _np_dtype(mode)
    wt_all = np.empty((NCORES, nsplit, NCHUNK, 128, VC), dtype=ndt)

    def fill(args):
        n, d = args
        # [VC, H] slice -> transpose to [H, VC] -> chunk rows of 128
        src32 = weight_stacked[d, n * VC:(n + 1) * VC, :].T  # [H, VC] view
        dst = wt_all[n, 0].reshape(D, H // 128, 128, VC)[d]  # [H//128, 128, VC]
        hi32 = np.ascontiguousarray(src32)
        np.copyto(dst.reshape(H, VC), hi32, casting="unsafe")
        if nsplit == 2:
            lo = (hi32 - dst.reshape(H, VC).astype(np.float32)).astype(ndt)
            np.copyto(wt_all[n, 1].reshape(D, H // 128, 128, VC)[d].reshape(H, VC),
                      lo, casting="unsafe")

    with ThreadPoolExecutor(max_workers=16) as ex:
        list(ex.map(fill, [(n, d) for n in range(NCORES) for d in range(D)]))
    return wt_all


def kernel(hidden_states, weight_stacked, indices, mode=None, _trace=False,
           _trace_kwargs=None):
    mode = mode or MODE
    hidden_states = np.asarray(hidden_states, dtype=np.float32)
    weight_stacked = np.asarray(weight_stacked, dtype=np.float32)
    indices = np.asarray(indices, dtype=np.int32)

    if mode not in _cache:
        _cache[mode] = _build(mode)
    nc = _cache[mode]

    hmt = _prep_hmt(hidden_states, indices, mode)
    wt_all = _prep_wt(weight_stacked, mode)

    in_maps = [{"hmt": hmt, "wt": wt_all[n]} for n in range(NCORES)]
    res = bass_utils.run_bass_kernel_spmd(
        nc, in_maps, core_ids=list(range(NCORES)),
        trace=_trace, **(_trace_kwargs or {}),
    )
    out = np.concatenate([res.results[n]["out"] for n in range(NCORES)], axis=1)
    if _trace:
        kernel._last_results = res
    return out


# revision 8
# speedup vs baseline: 1.2672x; 1.2672x over previous
"""Bass/Trainium2 kernel for nn_LogitsProcessorWithPacked.

Computes out[t, :] = weight_stacked[indices[t]] @ hidden_states[t]
 (T=64 tokens, H=2048 hidden, V=32000 vocab, D=4 stacked deltas, fp32).

Strategy (per sharding hint): shard weight_stacked along the vocab dim
across the 8 cores (column-parallel LM head, 4000 vocab rows per core),
replicate hidden_states/indices, gather partial logits along vocab on the
host.

Host-side prep (cheap, O(bytes) layout work only — all FLOPs run on device):
  * indices -> per-delta masks; build masked-transposed hidden HmT
    [D*H, T] and pack it into the SBUF partition layout [128, 64*64].
  * per-core weight slice [D, 4000, H] -> transposed chunk-major layout
    [64, 128, 4000] (chunk c = (d, h-block), partition p = h within block)
    so each chunk DMA is fully contiguous 16KB-per-partition lines.

Device kernel (per core): stream the 131MB of W^T through SBUF with
double-buffered 4MB DMAs; for each chunk c the PE accumulates
  acc_j[t, v'] += HmT_chunk_c.T @ WT_chunk_c[:, j-block]
into 8 PSUM-bank accumulators (one per 500-wide vocab block), fp32 PSUM.
This is memory(HBM)-bound: ~131MB / ~3.5e11 B/s ~ 380us per core.
"""

import numpy as np
from concurrent.futures import ThreadPoolExecutor

from concourse import bacc, mybir, tile
from concourse import bass_utils

# Problem constants (hardcoded per contract)
T = 64          # tokens
H = 2048        # hidden
V = 32000       # vocab
D = 4           # stacked deltas
NCORES = 8
VC = V // NCORES            # 4000 vocab rows per core
NCHUNK = D * H // 128       # 64 chunks of 128 contraction rows
VBLK = 500                  # vocab block per PSUM bank (500*4B = 2000B <= 2KB bank)
NJ = VC // VBLK             # 8 vocab blocks
NJ2 = NJ // 2               # psum accumulators (2 vocab blocks share one, via
                            # PE column-tiling: col groups 0-63 / 64-127)

# chunks per DMA / weight buffering, per dtype size: 4MB transfers, triple
# buffered (measured best: 343us/core for f32r; 8MB x depth-2 measured 434us
# — too few transfers in flight exposes the ~2us per-DMA completion latency)
_DMA_PLAN = {4: (2, 3), 2: (4, 3)}  # dtype bytes -> (CPD, WBUFS)

# Numeric mode: "f32" exact (PE 4 cyc/row), "f32r" full-rate fp32 (HW reduced
# precision), "bf16x3"/"f16x3" hi/lo-split (3 products, ~1e-5 rel err,
# fp32-rate memory), "bf16"/"f16" single-pass (half memory traffic).
MODE = "f32r"

_cache = {}


def _mm_dtype(mode):
    return {
        "f32": mybir.dt.float32,
        "f32r": mybir.dt.float32r,
        "bf16": mybir.dt.bfloat16,
        "bf16x3": mybir.dt.bfloat16,
        "f16": mybir.dt.float16,
        "f16x3": mybir.dt.float16,
    }[mode]


def _nsplit(mode):
    return 2 if mode in ("bf16x3", "f16x3") else 1


def _build(mode):
    """Build + compile the per-core Bass module (SPMD: same NEFF, 8 cores)."""
    dt = _mm_dtype(mode)
    f32 = mybir.dt.float32
    nsplit = _nsplit(mode)  # hi(/lo) weight streams

    nc = bacc.Bacc("TRN2", target_bir_lowering=False, debug=False,
                   num_devices=NCORES)

    # hmt packs nsplit copies (hi, lo) of the masked-transposed hidden
    hmt_d = nc.dram_tensor("hmt", [128, nsplit, NCHUNK * T], dt,
                           kind="ExternalInput")
    wt_d = nc.dram_tensor("wt", [nsplit, NCHUNK, 128, VC], dt,
                          kind="ExternalInput")
    out_d = nc.dram_tensor("out", [T, VC], f32, kind="ExternalOutput")

    CPD, WBUFS = _DMA_PLAN[4 if dt in (f32, mybir.dt.float32r) else 2]

    with tile.TileContext(nc) as tc:
        with (
            tc.tile_pool(name="const", bufs=1) as const_pool,
            tc.tile_pool(name="wpool", bufs=WBUFS) as wpool,
            tc.tile_pool(name="accp", bufs=1, space="PSUM") as accp,
            tc.tile_pool(name="opool", bufs=1) as opool,
        ):
            hmt_sb = const_pool.tile([128, nsplit, NCHUNK * T], dt, name="hmt_sb")
            nc.sync.dma_start(hmt_sb[:], hmt_d[:])

            # 8 PSUM-bank accumulators, one per 500-wide vocab block.
            # (PE column-tiling two blocks into one [128, VBLK] bank was tried
            # and is rejected by this toolchain: walrus asserts
            # s3d3_mm_valid_dst_partition for matmul dst base_partition=64.)
            accs = [
                accp.tile([T, VBLK], f32, tag=f"acc{j}", name=f"acc{j}")
                for j in range(NJ)
            ]
            out_sb = opool.tile([T, VC], f32, name="out_sb")

            n_mm = NCHUNK * nsplit  # accumulation group length per acc
            for s in range(nsplit):
                for cc in range(NCHUNK // CPD):
                    wt_t = wpool.tile([128, CPD, VC], dt, tag="wt", name="wt_t")
                    nc.sync.dma_start(
                        wt_t[:],
                        wt_d[s, cc * CPD:(cc + 1) * CPD].rearrange("k p v -> p k v"),
                    )
                    for k in range(CPD):
                        c = cc * CPD + k
                        mi = s * NCHUNK + c
                        for j in range(NJ):
                            rhs = wt_t[:, k, j * VBLK:(j + 1) * VBLK]
                            if nsplit == 2 and s == 0:
                                # products 1+2: (hmt_hi + hmt_lo) x wt_hi
                                for part in range(2):
                                    nc.tensor.matmul(
                                        accs[j][:],
                                        lhsT=hmt_sb[:, part, c * T:(c + 1) * T],
                                        rhs=rhs,
                                        start=(c == 0 and part == 0),
                                        stop=False,
                                    )
                            else:
                                # f32/f32r/bf16/f16: one product per chunk.
                                # x3 modes s==1: product 3: hmt_hi x wt_lo
                                nc.tensor.matmul(
                                    accs[j][:],
                                    lhsT=hmt_sb[:, 0, c * T:(c + 1) * T],
                                    rhs=rhs,
                                    start=(mi == 0),
                                    stop=(mi == n_mm - 1),
                                )
            for j in range(NJ):
                nc.vector.tensor_copy(out_sb[:, j * VBLK:(j + 1) * VBLK], accs[j][:])
            nc.sync.dma_start(out_d[:], out_sb[:])

    nc.compile()
    return nc


def _np_dtype(mode):
    if mode in ("bf16", "bf16x3"):
        import ml_dtypes
        return ml_dtypes.bfloat16
    if mode in ("f16", "f16x3"):
        return np.float16
    return np.float32


def _prep_hmt(hidden_states, indices, mode):
    """[128, nsplit, NCHUNK*T]: masked transposed hidden in partition layout."""
    masks = (indices[None, :] == np.arange(D, dtype=np.int32)[:, None])  # [D, T]
    # HmT[d*H + h, t] = H[t, h] * mask[d, t]
    hmt = (hidden_states.T[None, :, :] * masks[:, None, :]).reshape(D * H, T)
    # chunk-major partition packing: [NCHUNK, 128, T] -> [128, NCHUNK*T]
    packed32 = np.ascontiguousarray(
        hmt.reshape(NCHUNK, 128, T).transpose(1, 0, 2)
    ).reshape(128, NCHUNK * T)
    nsplit = _nsplit(mode)
    ndt = _np_dtype(mode)
    out = np.zeros((128, nsplit, NCHUNK * T), dtype=ndt)
    hi = packed32.astype(ndt)
    out[:, 0] = hi
    if nsplit == 2:
        out[:, 1] = (packed32 - hi.astype(np.float32)).astype(ndt)
    return out


def _prep_wt(weight_stacked, mode):
    """[NCORES][nsplit, NCHUNK, 128, VC] transposed chunk-major weight shards."""
    nsplit = _nsplit(mode)
    ndt = _np_dtype(mode)
    wt_all = np.empty((NCORES, nsplit, NCHUNK, 128, VC), dtype=ndt)

    def fill(args):
        n, d = args
        # [VC, H] slice -> transpose to [H, VC] -> chunk rows of 128
        src32 = weight_stacked[d, n * VC:(n + 1) * VC, :].T  # [H, VC] view
        dst = wt_all[n, 0].reshape(D, H // 128, 128, VC)[d]  # [H//128, 128, VC]
        hi32 = np.ascontiguousarray(src32)
        np.copyto(dst.reshape(H, VC), hi32, casting="unsafe")
        if nsplit == 2:
            lo = (hi32 - dst.reshape(H, VC).astype(np.float32)).astype(ndt)
            np.copyto(wt_all[n, 1].reshape(D, H // 128, 128, VC)[d].reshape(H, VC),
                      lo, casting="unsafe")

    with ThreadPoolExecutor(max_workers=16) as ex:
        list(ex.map(fill, [(n, d) for n in range(NCORES) for d in range(D)]))
    return wt_all


def kernel(hidden_states, weight_stacked, indices, mode=None, _trace=False,
           _trace_kwargs=None):
    mode = mode or MODE
    hidden_states = np.asarray(hidden_states, dtype=np.float32)
    weight_stacked = np.asarray(weight_stacked, dtype=np.float32)
    indices = np.asarray(indices, dtype=np.int32)

    if mode not in _cache:
        _cache[mode] = _build(mode)
    nc = _cache[mode]

    hmt = _prep_hmt(hidden_states, indices, mode)
    wt_all = _prep_wt(weight_stacked, mode)

    in_maps = [{"hmt": hmt, "wt": wt_all[n]} for n in range(NCORES)]
    res = bass_utils.run_bass_kernel_spmd(
        nc, in_maps, core_ids=list(range(NCORES)),
        trace=_trace, **(_trace_kwargs or {}),
    )
    out = np.concatenate([res.results[n]["out"] for n in range(NCORES)], axis=1)
    if _trace:
        kernel._last_results = res
    return out
